# revision 11
# baseline (speedup 1.0000x reference)
"""Entropic OT loss (CLIP-style) on 8 trn2 NeuronCores — Bass/Tile SPMD kernel.

Math (faithful to the reference's quirks):
  L = img @ txt.T                       (N=4096, D=512)
  For M1 = 1-L and M2 = 1-L.T:
    K = exp(-M/0.01);  Kinv = 1.0/K     (reference computes the reciprocal)
    5 Sinkhorn iterations:  v = b/(K.T@u);  u = Kinv@v
    P = u[:,None]*K*v[:,None]           (quirk: v indexed by ROW)
    CE = mean_i [ logsumexp_j P[i,j] - P[i,i] ]   (labels are arange)
  loss = (CE1+CE2)/2

Sharding (column shard, N/8 = 512 columns per core):
  Core c computes X = L1[:, jc] = img @ txt_c.T and Y = L2[:, jc] = txt @ img_c.T.
  K1col = exp(100*X-100)     -> rhs of the v1-update GEMV (contract all rows)
  Kinv1T = 1/K2col           -> rhs of the u1-update GEMV (Kinv1.T row-shard == 1/K2 col-shard)
  (and symmetrically for problem 2). Each GEMV produces a local 512-chunk of the
  length-4096 vector; an AllGather rebuilds the full vector for the next GEMV.
  The cross-entropy reduces with one batched AllReduce of the row-sum vectors
  plus a scalar AllReduce.

The computed loss is NaN (matching the reference bit-for-bit in the only way
that matters: exp(-M/0.01) underflows fp32, 1/K overflows to inf, and the
Sinkhorn iterations NaN-poison P; jax's log_softmax then yields NaN).

Host-side work is limited to data marshaling: dtype cast to bf16, transpose,
and per-core slicing. All FLOPs of the algorithm run on the NeuronCores.
"""

import os
import numpy as np

import concourse.bacc as bacc
import concourse.mybir as mybir
import concourse.tile as tile
from concourse.bass_utils import run_bass_kernel_spmd

F32 = mybir.dt.float32
BF16 = mybir.dt.bfloat16
AF = mybir.ActivationFunctionType
NP_BF16 = mybir.dt.np(BF16)

N = 4096          # batch
D = 512           # feature dim
NCORES = 8
S = N // NCORES   # 512 columns per core
NT = N // 128     # 32 row tiles
ND = D // 128     # 4 contraction tiles
NCH = 8           # i-chunks of 4 row-tiles each in the big matmuls
REG = 0.01
N_ITERS = 5
SCALE = 1.0 / REG         # 100.0
INV_N = 1.0 / N           # 1/4096 (exact in bf16)
HALF_INV_N = 1.0 / (2 * N)


def _build_program():
    nc = bacc.Bacc("TRN2", target_bir_lowering=False, debug=False,
                   num_devices=NCORES)

    imgT_d = nc.dram_tensor("imgT", [D, N], BF16, kind="ExternalInput").ap()
    txtT_d = nc.dram_tensor("txtT", [D, N], BF16, kind="ExternalInput").ap()
    rhsX_d = nc.dram_tensor("rhsX", [D, S], BF16, kind="ExternalInput").ap()
    rhsY_d = nc.dram_tensor("rhsY", [D, S], BF16, kind="ExternalInput").ap()
    iln_d = nc.dram_tensor("iln", [S, D], BF16, kind="ExternalInput").ap()
    tln_d = nc.dram_tensor("tln", [S, D], BF16, kind="ExternalInput").ap()
    loss_d = nc.dram_tensor("loss", [1, 1], F32, kind="ExternalOutput").ap()

    with tile.TileContext(nc) as tc:
        with (
            tc.tile_pool(name="kmat", bufs=1) as kpool,
            tc.tile_pool(name="sb", bufs=1) as sb,
            tc.tile_pool(name="win", bufs=10) as winp,
            tc.tile_pool(name="vec", bufs=2) as vec,
            tc.tile_pool(name="scr", bufs=2) as scr,
            tc.tile_pool(name="dram", bufs=2, space="DRAM") as dram,
        ):
            one_ap = nc.const_aps.tensor(1.0, (128, 1))

            # ---- constants on the ACT engine ----
            bias_m100 = sb.tile([128, 1], F32, tag="bm100")
            nc.scalar.mul(bias_m100[:], one_ap, -SCALE)

            # ---- warmups (no deps; scheduled immediately) ----
            # PE: ~8us of dummy matmuls flips the HAM clock-gate to 8/8 before
            # the real matmuls arrive
            wscr = sb.tile([128, 512], BF16, tag="wscr")
            nc.gpsimd.memset(wscr[:], 0.0)
            with tc.tile_pool(name="pswarm", bufs=1, space="PSUM") as pswarm:
                wps = pswarm.tile([1, 512], F32, tag="wps")
                for r in range(20):
                    nc.tensor.matmul(wps[:], wscr[:, 0:1], wscr[:],
                                     start=(r == 0), stop=(r == 19))
            # collectives: first AG/AR pays ~20us of ncfw cold start; absorb it
            # here, overlapped with the input DMAs and matmuls
            wag_in = dram.tile([1, 16], F32, tag="wagin")
            wag_out = dram.tile([NCORES, 16], F32, tag="wagout")
            war_out = dram.tile([1, 16], F32, tag="warout")
            nc.gpsimd.collective_compute(
                "AllGather", mybir.AluOpType.bypass,
                ins=[wag_in[:].opt()], outs=[wag_out[:].opt()],
                replica_groups=[list(range(NCORES))])
            nc.gpsimd.collective_compute(
                "AllReduce", mybir.AluOpType.add,
                ins=[wag_in[:].opt()], outs=[war_out[:].opt()],
                replica_groups=[list(range(NCORES))])

            # K matrices, column-sharded, [row-tile t at free 512t:512t+512]
            k1 = kpool.tile([128, NT * S], BF16, tag="k1")
            k2 = kpool.tile([128, NT * S], BF16, tag="k2")
            ki1 = kpool.tile([128, NT * S], BF16, tag="ki1")  # Kinv1T col = 1/K2col
            ki2 = kpool.tile([128, NT * S], BF16, tag="ki2")  # Kinv2T col = 1/K1col

            # ---- rhs (stationary side of the big matmuls) + ldiag inputs ----
            rhsX = [sb.tile([128, S], BF16, tag=f"rx{dt}", name=f"rhsX{dt}") for dt in range(ND)]
            rhsY = [sb.tile([128, S], BF16, tag=f"ry{dt}", name=f"rhsY{dt}") for dt in range(ND)]
            for dt in range(ND):
                nc.sync.dma_start(rhsX[dt][:], rhsX_d[128 * dt:128 * (dt + 1), :])
                nc.sync.dma_start(rhsY[dt][:], rhsY_d[128 * dt:128 * (dt + 1), :])

            # diag(L) for local rows: sum_d img[i,:]*txt[i,:]  -> [128, 4]
            ldiag = sb.tile([128, ND], F32, tag="ldiag")
            for q in range(ND):
                ilq = scr.tile([128, D], BF16, tag="ilq")
                tlq = scr.tile([128, D], BF16, tag="tlq")
                nc.sync.dma_start(ilq[:], iln_d[128 * q:128 * (q + 1), :])
                nc.sync.dma_start(tlq[:], tln_d[128 * q:128 * (q + 1), :])
                prod = scr.tile([128, D], F32, tag="ldprod")
                nc.vector.tensor_mul(prod[:], ilq[:], tlq[:])
                nc.vector.reduce_sum(ldiag[:, q:q + 1], prod[:],
                                     axis=mybir.AxisListType.X)
            # diag of K1 (== diag of K2): exp(100*ldiag - 100)
            kdiag = sb.tile([128, ND], F32, tag="kdiag")
            nc.scalar.activation(kdiag[:], ldiag[:], AF.Exp,
                                 bias=bias_m100[:], scale=SCALE)

            # ---- big matmuls + K production ----
            # bias for Kinv = exp(100 - 100*L)  (== 1/K up to fp rounding; the
            # reference's 1.0/K overflows to the same inf/0 garbage classes)
            bias_p100 = sb.tile([128, 1], F32, tag="bp100")
            nc.scalar.mul(bias_p100[:], one_ap, SCALE)
            with tc.tile_pool(name="psmm", bufs=2, space="PSUM") as psmm:
                with nc.allow_low_precision("bf16 K matrices; output is NaN"):
                    for lhs_d, kt, kit in ((imgT_d, k1, ki2), (txtT_d, k2, ki1)):
                        for ic in range(NCH):
                            win = []
                            for dt in range(ND):
                                w = winp.tile([128, 512], BF16, tag="win", name=f"win_{dt}")
                                nc.sync.dma_start(
                                    w[:],
                                    lhs_d[128 * dt:128 * (dt + 1),
                                          512 * ic:512 * (ic + 1)])
                                win.append(w)
                            rhs = rhsX if kt is k1 else rhsY
                            # one 4-bank PSUM tile holds the whole i-chunk so
                            # the exps amortize ACT per-op overhead over 2048
                            ps = psmm.tile([128, 4 * S], F32, tag="mm")
                            for tin in range(4):
                                for dt in range(ND):
                                    nc.tensor.matmul(
                                        ps[:, S * tin:S * (tin + 1)],
                                        win[dt][:, 128 * tin:128 * (tin + 1)],
                                        rhs[dt][:],
                                        start=(dt == 0), stop=(dt == ND - 1))
                            csl = slice(S * 4 * ic, S * 4 * (ic + 1))
                            nc.scalar.activation(kt[:, csl], ps[:], AF.Exp,
                                                 bias=bias_m100[:], scale=SCALE)
                            nc.scalar.activation(kit[:, csl], ps[:], AF.Exp,
                                                 bias=bias_p100[:], scale=-SCALE)

            # ---- Sinkhorn ----
            ufull = [None, None]  # [128, 32] bf16, element [p,t] = u[128t+p]
            vfull = [None, None]
            for p in range(2):
                uf = vec.tile([128, NT], BF16, tag=f"uf{p}", name=f"uf0_{p}")
                nc.vector.memset(uf[:], INV_N)
                ufull[p] = uf

            kmat = (k1, k2)
            kinvT = (ki1, ki2)
            sb_in_last = [None, None]
            ub_in_last = [None, None]
            with tc.tile_pool(name="psg", bufs=4, space="PSUM") as psg, \
                    nc.allow_low_precision("bf16 sinkhorn vectors; output is NaN"):
                for it in range(N_ITERS):
                    # part A: both problems' GEMVs + DMA-out + AllGather first,
                    # so no DMA-in (which waits on an AG) blocks another
                    # problem's DMA-out in the same engine FIFO
                    s_outs = []
                    for p in range(2):
                        s_ps = psg.tile([1, S], F32, tag="gemv",
                                        name=f"sps_{p}_{it}")
                        for t in range(NT):
                            nc.tensor.matmul(
                                s_ps[:], ufull[p][:, t:t + 1],
                                kmat[p][:, S * t:S * (t + 1)],
                                start=(t == 0), stop=(t == NT - 1))
                        s_sb = scr.tile([1, S], F32, tag=f"ssb{p}",
                                        name=f"ssb_{p}_{it}")
                        nc.scalar.copy(s_sb[:], s_ps[:])
                        sb_in = dram.tile([1, S], F32, tag=f"sin{p}",
                                          name=f"sin_{p}_{it}")
                        sb_out = dram.tile([NCORES, S], F32, tag=f"sout{p}",
                                           name=f"sout_{p}_{it}")
                        nc.sync.dma_start(sb_in[:], s_sb[:])
                        nc.gpsimd.collective_compute(
                            "AllGather", mybir.AluOpType.bypass,
                            ins=[sb_in[:].opt()], outs=[sb_out[:].opt()],
                            replica_groups=[list(range(NCORES))])
                        s_outs.append(sb_out)
                        if it == N_ITERS - 1:
                            sb_in_last[p] = sb_in
                    # part B: gather results back and build v
                    for p in range(2):
                        sf = vec.tile([128, NT], F32, tag=f"sf{p}",
                                      name=f"sf_{p}_{it}")
                        nc.sync.dma_start(
                            sf[:],
                            s_outs[p][:].rearrange("a b -> (a b)")
                                        .rearrange("(t q) -> q t", q=128))
                        vr = vec.tile([128, NT], F32, tag=f"vr{p}",
                                      name=f"vr_{p}_{it}")
                        nc.vector.reciprocal(vr[:], sf[:])
                        vf = vec.tile([128, NT], BF16, tag=f"vf{p}",
                                      name=f"vf_{p}_{it}")
                        nc.vector.tensor_scalar_mul(vf[:], vr[:], INV_N)
                        vfull[p] = vf
                    # ---- u = Kinv @ v ----
                    u_outs = []
                    for p in range(2):
                        u_ps = psg.tile([1, S], F32, tag="gemv",
                                        name=f"ups_{p}_{it}")
                        for t in range(NT):
                            nc.tensor.matmul(
                                u_ps[:], vfull[p][:, t:t + 1],
                                kinvT[p][:, S * t:S * (t + 1)],
                                start=(t == 0), stop=(t == NT - 1))
                        u_sb = scr.tile([1, S], F32, tag=f"usb{p}",
                                        name=f"usb_{p}_{it}")
                        nc.scalar.copy(u_sb[:], u_ps[:])
                        ub_in = dram.tile([1, S], F32, tag=f"uin{p}",
                                          name=f"uin_{p}_{it}")
                        ub_out = dram.tile([NCORES, S], F32, tag=f"uout{p}",
                                           name=f"uout_{p}_{it}")
                        nc.sync.dma_start(ub_in[:], u_sb[:])
                        nc.gpsimd.collective_compute(
                            "AllGather", mybir.AluOpType.bypass,
                            ins=[ub_in[:].opt()], outs=[ub_out[:].opt()],
                            replica_groups=[list(range(NCORES))])
                        u_outs.append(ub_out)
                        if it == N_ITERS - 1:
                            ub_in_last[p] = ub_in
                    for p in range(2):
                        uff = vec.tile([128, NT], F32, tag=f"uff{p}",
                                       name=f"uff_{p}_{it}")
                        nc.sync.dma_start(
                            uff[:],
                            u_outs[p][:].rearrange("a b -> (a b)")
                                        .rearrange("(t q) -> q t", q=128))
                        uf = vec.tile([128, NT], BF16, tag=f"uf{p}",
                                      name=f"uf_{p}_{it}")
                        nc.vector.tensor_copy(uf[:], uff[:])
                        ufull[p] = uf

            # ---- loss ----
            # row sums s[i] = sum_j exp(P[i,j]); P = (u*v)[i] * K[i,j]
            s_acc = [None, None]
            with nc.allow_low_precision("bf16 P tiles; output is NaN"):
                for p in range(2):
                    cfull = sb.tile([128, NT], F32, tag=f"cf{p}")
                    nc.vector.tensor_mul(cfull[:], ufull[p][:], vfull[p][:])
                    sa = sb.tile([128, NT], F32, tag=f"sa{p}")
                    s_acc[p] = sa
                    for t in range(NT):
                        pt = scr.tile([128, S], BF16, tag="pt")
                        nc.vector.tensor_scalar_mul(
                            pt[:], kmat[p][:, S * t:S * (t + 1)],
                            cfull[:, t:t + 1])
                        ptrash = scr.tile([128, S], BF16, tag="ptrash")
                        nc.scalar.activation(ptrash[:], pt[:], AF.Exp,
                                             accum_out=sa[:, t:t + 1])

            # batched AllReduce of both problems' row-sum vectors
            sar_in = dram.tile([2, N], F32, tag="sarin")
            sar_out = dram.tile([2, N], F32, tag="sarout")
            for p in range(2):
                nc.sync.dma_start(
                    sar_in[p, :].rearrange("(t q) -> q t", q=128), s_acc[p][:])
            nc.gpsimd.collective_compute(
                "AllReduce", mybir.AluOpType.add,
                ins=[sar_in[:].opt()], outs=[sar_out[:].opt()],
                replica_groups=[list(range(NCORES))])

            total = sb.tile([128, 1], F32, tag="total")
            nc.vector.memset(total[:], 0.0)
            for p in range(2):
                sfull = scr.tile([128, NT], F32, tag="sfull")
                nc.sync.dma_start(
                    sfull[:], sar_out[p, :].rearrange("(t q) -> q t", q=128))
                logs = scr.tile([128, NT], F32, tag="logs")
                nc.scalar.activation(logs[:], sfull[:], AF.Ln)
                red = scr.tile([128, 1], F32, tag="red")
                nc.vector.reduce_sum(red[:], logs[:], axis=mybir.AxisListType.X)
                # every core computes the identical full sum; the final scalar
                # AllReduce adds 8 copies, so scale by 1/8 here
                sc = scr.tile([128, 1], F32, tag="sc")
                nc.vector.tensor_scalar_mul(sc[:], red[:], 1.0 / NCORES)
                nc.vector.tensor_add(total[:], total[:], sc[:])

                # diagonal term: P[i,i] = u[i]*v[i]*Kdiag[i] for the local rows
                # (rebuilt from the final-iteration local chunks in DRAM)
                s128 = scr.tile([128, ND], F32, tag="s128")
                u128 = scr.tile([128, ND], F32, tag="u128")
                nc.sync.dma_start(
                    s128[:], sb_in_last[p][:].rearrange("a b -> (a b)")
                                             .rearrange("(t q) -> q t", q=128))
                nc.sync.dma_start(
                    u128[:], ub_in_last[p][:].rearrange("a b -> (a b)")
                                             .rearrange("(t q) -> q t", q=128))
                vr128 = scr.tile([128, ND], F32, tag="vr128")
                nc.vector.reciprocal(vr128[:], s128[:])
                v128 = scr.tile([128, ND], F32, tag="v128")
                nc.vector.tensor_scalar_mul(v128[:], vr128[:], INV_N)
                cd = scr.tile([128, ND], F32, tag="cd")
                nc.vector.tensor_mul(cd[:], u128[:], v128[:])
                dt_ = scr.tile([128, ND], F32, tag="dt")
                nc.vector.tensor_mul(dt_[:], cd[:], kdiag[:])
                redd = scr.tile([128, 1], F32, tag="redd")
                nc.vector.reduce_sum(redd[:], dt_[:], axis=mybir.AxisListType.X)
                nc.vector.tensor_sub(total[:], total[:], redd[:])

            # partition sum via ones.T @ total (fp32 matmul, 1 column)
            with tc.tile_pool(name="pssc", bufs=1, space="PSUM") as pssc:
                tot_ps = pssc.tile([1, 1], F32, tag="tot")
                nc.tensor.matmul(tot_ps[:], one_ap, total[:],
                                 start=True, stop=True)
                tot_sb = sb.tile([1, 1], F32, tag="totsb")
                nc.scalar.copy(tot_sb[:], tot_ps[:])

            tar_in = dram.tile([1, 1], F32, tag="tarin")
            tar_out = dram.tile([1, 1], F32, tag="tarout")
            nc.gpsimd.dma_start(tar_in[:], tot_sb[:])
            nc.gpsimd.collective_compute(
                "AllReduce", mybir.AluOpType.add,
                ins=[tar_in[:].opt()], outs=[tar_out[:].opt()],
                replica_groups=[list(range(NCORES))])
            fin = sb.tile([1, 1], F32, tag="fin")
            nc.sync.dma_start(fin[:], tar_out[:])
            out_sb = sb.tile([1, 1], F32, tag="outsb")
            nc.scalar.mul(out_sb[:], fin[:], HALF_INV_N)
            nc.sync.dma_start(loss_d, out_sb[:])

    nc.compile()
    return nc


_NC_CACHE = {}


def _get_program():
    if "nc" not in _NC_CACHE:
        _NC_CACHE["nc"] = _build_program()
    return _NC_CACHE["nc"]


def kernel(all_image_features, all_text_features, labels=None, **_unused):
    img = np.asarray(all_image_features, dtype=np.float32)
    txt = np.asarray(all_text_features, dtype=np.float32)
    assert img.shape == (N, D) and txt.shape == (N, D)

    # host-side marshaling only: bf16 cast + transpose + per-core slicing
    imgT = np.ascontiguousarray(img.T).astype(NP_BF16)
    txtT = np.ascontiguousarray(txt.T).astype(NP_BF16)
    img_bf = img.astype(NP_BF16)
    txt_bf = txt.astype(NP_BF16)

    in_maps = []
    for c in range(NCORES):
        sl = slice(S * c, S * (c + 1))
        in_maps.append({
            "imgT": imgT,
            "txtT": txtT,
            "rhsX": np.ascontiguousarray(txtT[:, sl]),
            "rhsY": np.ascontiguousarray(imgT[:, sl]),
            "iln": np.ascontiguousarray(img_bf[sl, :]),
            "tln": np.ascontiguousarray(txt_bf[sl, :]),
        })

    nc = _get_program()
    trace = bool(int(os.environ.get("OT_KERNEL_TRACE", "0")))
    res = run_bass_kernel_spmd(nc, in_maps, list(range(NCORES)), trace=trace)
    if trace:
        _NC_CACHE["last_exec_time_ns"] = res.exec_time_ns
        _NC_CACHE["last_results"] = res
    loss = np.float32(res.results[0]["loss"][0, 0])
    return np.asarray(loss, dtype=np.float32).reshape(())


# revision 14
# speedup vs baseline: 1.0661x; 1.0661x over previous
"""Entropic OT loss (CLIP-style) on 8 trn2 NeuronCores — Bass/Tile SPMD kernel.

Math (faithful to the reference's quirks):
  L = img @ txt.T                       (N=4096, D=512)
  For M1 = 1-L and M2 = 1-L.T:
    K = exp(-M/0.01);  Kinv = 1.0/K
    5 Sinkhorn iterations:  v = b/(K.T@u);  u = Kinv@v
    P = u[:,None]*K*v[:,None]           (quirk: v indexed by ROW)
    CE = mean_i [ logsumexp_j P[i,j] - P[i,i] ]   (labels are arange)
  loss = (CE1+CE2)/2

Sharding: row/col hybrid, N/8 = 512 rows (or cols) per core.
  Krow_p  = exp(100*L_p[rows_c,:]-100)      [512,4096] row shard
  kiT_p   = exp(100-100*L_other[:,rows_c])  [4096,512] = Kinv_p.T col shard
  s-GEMV  contracts the LOCAL rows of Krow (lhsT = the locally produced
          u-chunk) -> one AllReduce of the length-4096 partial sums per
          iteration per problem.  v = (1/N)/s is computed post-reduce.
  u-GEMV  contracts all 4096 rows of kiT with the replicated v -> the
          u-chunk stays LOCAL (no collective on the u hop).
  The cross-entropy is row-local (full rows of Krow on-core): only a scalar
  AllReduce at the end.

The computed loss is NaN, matching the reference: exp(-M/0.01) underflows
fp32, 1/K overflows to inf, and the Sinkhorn iterations NaN-poison P; the
log_softmax then yields NaN.  Host-side work is limited to data marshaling
(bf16 cast, transpose, slicing, index masks); all FLOPs run on-device.
"""

import os
import numpy as np

import concourse.bacc as bacc
import concourse.mybir as mybir
import concourse.tile as tile
from concourse.bass_utils import run_bass_kernel_spmd

F32 = mybir.dt.float32
BF16 = mybir.dt.bfloat16
AF = mybir.ActivationFunctionType
NP_BF16 = mybir.dt.np(BF16)

N = 4096          # batch
D = 512           # feature dim
NCORES = 8
S = N // NCORES   # 512 rows per core
NT = N // 128     # 32 tiles over the global 4096 dim
ND = D // 128     # 4 tiles over the 512-dim (d or local rows)
REG = 0.01
N_ITERS = 5
SCALE = 1.0 / REG         # 100.0
INV_N = 1.0 / N
HALF_INV_N = 1.0 / (2 * N)
RG = [list(range(NCORES))]


def _build_program():
    nc = bacc.Bacc("TRN2", target_bir_lowering=False, debug=False,
                   num_devices=NCORES)

    imgT_d = nc.dram_tensor("imgT", [D, N], BF16, kind="ExternalInput").ap()
    txtT_d = nc.dram_tensor("txtT", [D, N], BF16, kind="ExternalInput").ap()
    # local transposed feature blocks (columns 512c:512c+512 of imgT/txtT)
    ilocT_d = nc.dram_tensor("ilocT", [D, S], BF16, kind="ExternalInput").ap()
    tlocT_d = nc.dram_tensor("tlocT", [D, S], BF16, kind="ExternalInput").ap()
    # local feature rows, natural layout (for diag(L))
    iln_d = nc.dram_tensor("iln", [S, D], BF16, kind="ExternalInput").ap()
    tln_d = nc.dram_tensor("tln", [S, D], BF16, kind="ExternalInput").ap()
    # one-hot masks: mask q selects column 4c+q of a [128, 32] full-vector tile
    vmask_d = nc.dram_tensor("vmask", [128, ND * NT], F32,
                             kind="ExternalInput").ap()
    loss_d = nc.dram_tensor("loss", [1, 1], F32, kind="ExternalOutput").ap()

    with tile.TileContext(nc) as tc:
        with (
            tc.tile_pool(name="kmat", bufs=1) as kpool,
            tc.tile_pool(name="sb", bufs=1) as sb,
            tc.tile_pool(name="win", bufs=8) as winp,
            tc.tile_pool(name="vec", bufs=2) as vec,
            tc.tile_pool(name="scr", bufs=2) as scr,
            tc.tile_pool(name="dram", bufs=2, space="DRAM") as dram,
        ):
            one_ap = nc.const_aps.tensor(1.0, (128, 1))

            # ---- constants on the ACT engine ----
            bias_m100 = sb.tile([128, 1], F32, tag="bm100")
            nc.scalar.mul(bias_m100[:], one_ap, -SCALE)
            bias_p100 = sb.tile([128, 1], F32, tag="bp100")
            nc.scalar.mul(bias_p100[:], one_ap, SCALE)

            # ---- warmups (no deps; scheduled immediately) ----
            wscr = sb.tile([128, 512], BF16, tag="wscr")
            nc.gpsimd.memset(wscr[:], 0.0)
            with tc.tile_pool(name="pswarm", bufs=1, space="PSUM") as pswarm:
                wps = pswarm.tile([1, 512], F32, tag="wps")
                for r in range(20):
                    nc.tensor.matmul(wps[:], wscr[:, 0:1], wscr[:],
                                     start=(r == 0), stop=(r == 19))
            wag_in = dram.tile([1, 16], F32, tag="wagin")
            wag_out = dram.tile([NCORES, 16], F32, tag="wagout")
            war_out = dram.tile([1, 16], F32, tag="warout")
            nc.gpsimd.collective_compute(
                "AllGather", mybir.AluOpType.bypass,
                ins=[wag_in[:].opt()], outs=[wag_out[:].opt()],
                replica_groups=RG)
            nc.gpsimd.collective_compute(
                "AllReduce", mybir.AluOpType.add,
                ins=[wag_in[:].opt()], outs=[war_out[:].opt()],
                replica_groups=RG)

            # K matrices (bf16, 4MB each):
            # krow[p]: [128, 4*4096], row-tile m at free m*4096+j
            # kiT[p]:  [128, 32*512], j-tile t at free 512*t
            krow = [kpool.tile([128, ND * N], BF16, tag=f"krow{p}",
                               name=f"krow{p}") for p in range(2)]
            kiT = [kpool.tile([128, NT * S], BF16, tag=f"ki{p}",
                              name=f"kiT{p}") for p in range(2)]

            # resident local transposed blocks
            ilocT = [sb.tile([128, S], BF16, tag=f"il{dt}", name=f"ilocT{dt}")
                     for dt in range(ND)]
            tlocT = [sb.tile([128, S], BF16, tag=f"tl{dt}", name=f"tlocT{dt}")
                     for dt in range(ND)]
            for dt in range(ND):
                nc.sync.dma_start(ilocT[dt][:],
                                  ilocT_d[128 * dt:128 * (dt + 1), :])
                nc.sync.dma_start(tlocT[dt][:],
                                  tlocT_d[128 * dt:128 * (dt + 1), :])
            vmask = sb.tile([128, ND * NT], F32, tag="vmask")
            nc.sync.dma_start(vmask[:], vmask_d)

            # diag(L) for local rows -> [128, 4]; kdiag = exp(100*ld - 100)
            ldiag = sb.tile([128, ND], F32, tag="ldiag")
            for q in range(ND):
                ilq = scr.tile([128, D], BF16, tag="ilq")
                tlq = scr.tile([128, D], BF16, tag="tlq")
                nc.sync.dma_start(ilq[:], iln_d[128 * q:128 * (q + 1), :])
                nc.sync.dma_start(tlq[:], tln_d[128 * q:128 * (q + 1), :])
                prod = scr.tile([128, D], F32, tag="ldprod")
                nc.vector.tensor_mul(prod[:], ilq[:], tlq[:])
                nc.vector.reduce_sum(ldiag[:, q:q + 1], prod[:],
                                     axis=mybir.AxisListType.X)
            kdiag = sb.tile([128, ND], F32, tag="kdiag")
            nc.scalar.activation(kdiag[:], ldiag[:], AF.Exp,
                                 bias=bias_m100[:], scale=SCALE)

            lowp = nc.allow_low_precision("bf16 K matrices; output is NaN")
            lowp.__enter__()

            # ---- row products: Krow_p = exp(100*Lrow - 100) ----
            # Lrow1 = img_loc @ txt.T  (lhsT = ilocT resident, rhs = txtT)
            # Lrow2 = txt_loc @ img.T  (lhsT = tlocT resident, rhs = imgT)
            with tc.tile_pool(name="psrow", bufs=2, space="PSUM") as psrow:
                for p, (lres, rstream) in enumerate(
                        ((ilocT, txtT_d), (tlocT, imgT_d))):
                    for jc2 in range(2):          # 2048-wide column chunks
                        rwin = []
                        for dt in range(ND):
                            w = winp.tile([128, 2048], BF16, tag="rwin",
                                          name=f"rwin{dt}", bufs=5)
                            nc.sync.dma_start(
                                w[:], rstream[128 * dt:128 * (dt + 1),
                                              2048 * jc2:2048 * (jc2 + 1)])
                            rwin.append(w)
                        for m in range(ND):
                            ps = psrow.tile([128, 2048], F32, tag="rps")
                            for jc in range(4):
                                for dt in range(ND):
                                    nc.tensor.matmul(
                                        ps[:, 512 * jc:512 * (jc + 1)],
                                        lres[dt][:, 128 * m:128 * (m + 1)],
                                        rwin[dt][:, 512 * jc:512 * (jc + 1)],
                                        start=(dt == 0), stop=(dt == ND - 1))
                            off = N * m + 2048 * jc2
                            nc.scalar.activation(
                                krow[p][:, off:off + 2048], ps[:], AF.Exp,
                                bias=bias_m100[:], scale=SCALE)

            # ---- Sinkhorn state ----
            # uloc_bf[p]: [128, 4] bf16, [q, m] = u_loc[128m+q]
            uloc_bf = [None, None]
            for p in range(2):
                u0 = vec.tile([128, ND], BF16, tag=f"u0{p}", name=f"u0_{p}")
                nc.vector.memset(u0[:], INV_N)
                uloc_bf[p] = u0
            sfull_last = [None, None]
            uloc_f_last = [None, None]

            def s_gemv_phase(it, psg):
                """s = (local rows of K).T @ u_loc -> AllReduce; v = 1/(N*s)."""
                ar_outs = []
                for p in range(2):
                    ar_in = dram.tile([1, N], F32, tag=f"arin{p}",
                                      name=f"arin_{p}_{it}")
                    ar_out = dram.tile([1, N], F32, tag=f"arout{p}",
                                       name=f"arout_{p}_{it}")
                    for jc in range(8):
                        ps = psg.tile([1, 512], F32, tag="gemv",
                                      name=f"sps_{p}_{it}_{jc}")
                        for m in range(ND):
                            nc.tensor.matmul(
                                ps[:], uloc_bf[p][:, m:m + 1],
                                krow[p][:, N * m + 512 * jc:
                                        N * m + 512 * (jc + 1)],
                                start=(m == 0), stop=(m == ND - 1))
                        s_sb = scr.tile([1, 512], F32, tag=f"ssb{p}",
                                        name=f"ssb_{p}_{it}_{jc}")
                        nc.scalar.copy(s_sb[:], ps[:])
                        nc.sync.dma_start(
                            ar_in[0:1, 512 * jc:512 * (jc + 1)], s_sb[:])
                    nc.gpsimd.collective_compute(
                        "AllReduce", mybir.AluOpType.add,
                        ins=[ar_in[:].opt()], outs=[ar_out[:].opt()],
                        replica_groups=RG)
                    ar_outs.append(ar_out)
                vfs = []
                for p in range(2):
                    sf = vec.tile([128, NT], F32, tag=f"sf{p}",
                                  name=f"sf_{p}_{it}")
                    nc.sync.dma_start(
                        sf[:],
                        ar_outs[p][:].rearrange("a b -> (a b)")
                                     .rearrange("(t q) -> q t", q=128))
                    vr = vec.tile([128, NT], F32, tag=f"vr{p}",
                                  name=f"vr_{p}_{it}")
                    nc.vector.reciprocal(vr[:], sf[:])
                    vf = vec.tile([128, NT], BF16, tag=f"vf{p}",
                                  name=f"vf_{p}_{it}")
                    nc.vector.tensor_scalar_mul(vf[:], vr[:], INV_N)
                    vfs.append(vf)
                    if it == N_ITERS - 1:
                        sfull_last[p] = sf
                return vfs

            def u_gemv_phase(it, psg, vfs):
                """u_loc = kiT.T @ v (local result; no collective)."""
                for p in range(2):
                    ups = psg.tile([1, S], F32, tag="gemv",
                                   name=f"ups_{p}_{it}")
                    for t in range(NT):
                        nc.tensor.matmul(
                            ups[:], vfs[p][:, t:t + 1],
                            kiT[p][:, S * t:S * (t + 1)],
                            start=(t == 0), stop=(t == NT - 1))
                    u_sb = scr.tile([1, S], F32, tag=f"usb{p}",
                                    name=f"usb_{p}_{it}")
                    nc.scalar.copy(u_sb[:], ups[:])
                    usc = dram.tile([1, S], F32, tag=f"usc{p}",
                                    name=f"usc_{p}_{it}")
                    nc.sync.dma_start(usc[:], u_sb[:])
                    uf = vec.tile([128, ND], F32, tag=f"uf{p}",
                                  name=f"uf_{p}_{it}")
                    nc.sync.dma_start(
                        uf[:],
                        usc[:].rearrange("a b -> (a b)")
                              .rearrange("(m q) -> q m", q=128))
                    ub = vec.tile([128, ND], BF16, tag=f"ub{p}",
                                  name=f"ub_{p}_{it}")
                    nc.vector.tensor_copy(ub[:], uf[:])
                    uloc_bf[p] = ub
                    if it == N_ITERS - 1:
                        uloc_f_last[p] = uf

            with tc.tile_pool(name="psg", bufs=4, space="PSUM") as psg:
                # iteration 0 s-phase first, so its AllReduce overlaps the
                # column-product matmuls below
                vfs0 = s_gemv_phase(0, psg)

                # ---- col products: kiT_p = exp(100 - 100*Lcol_other) ----
                # kiT1 needs Lcol2[:, rows_c] = txt @ img_loc.T
                #   (lhsT = txtT blocks streamed, rhs = ilocT resident)
                # kiT2 needs Lcol1[:, rows_c] = img @ txt_loc.T
                with tc.tile_pool(name="pscol", bufs=2, space="PSUM") as pscol:
                    for p, (lstream, rres) in enumerate(
                            ((txtT_d, ilocT), (imgT_d, tlocT))):
                        for ic in range(8):
                            cwin = []
                            for dt in range(ND):
                                w = winp.tile([128, 512], BF16, tag="cwin",
                                              name=f"cwin{dt}")
                                nc.sync.dma_start(
                                    w[:], lstream[128 * dt:128 * (dt + 1),
                                                  512 * ic:512 * (ic + 1)])
                                cwin.append(w)
                            for half in range(2):
                                ps = pscol.tile([128, 1024], F32, tag="cps")
                                for ti in range(2):
                                    tin = 2 * half + ti
                                    for dt in range(ND):
                                        nc.tensor.matmul(
                                            ps[:, 512 * ti:512 * (ti + 1)],
                                            cwin[dt][:, 128 * tin:
                                                      128 * (tin + 1)],
                                            rres[dt][:],
                                            start=(dt == 0),
                                            stop=(dt == ND - 1))
                                off = S * 4 * ic + 1024 * half
                                nc.scalar.activation(
                                    kiT[p][:, off:off + 1024], ps[:], AF.Exp,
                                    bias=bias_p100[:], scale=-SCALE)

                u_gemv_phase(0, psg, vfs0)
                for it in range(1, N_ITERS):
                    vfs = s_gemv_phase(it, psg)
                    u_gemv_phase(it, psg, vfs)

            # ---- loss (row-local) ----
            total = sb.tile([128, 1], F32, tag="total")
            nc.vector.memset(total[:], 0.0)
            for p in range(2):
                # v_loc extraction from the final s via one-hot masks
                sloc = scr.tile([128, ND], F32, tag="sloc")
                for q in range(ND):
                    tmpm = scr.tile([128, NT], F32, tag="tmpm")
                    nc.vector.tensor_mul(
                        tmpm[:], sfull_last[p][:],
                        vmask[:, NT * q:NT * (q + 1)])
                    nc.vector.reduce_sum(sloc[:, q:q + 1], tmpm[:],
                                         axis=mybir.AxisListType.X)
                vrl = scr.tile([128, ND], F32, tag="vrl")
                nc.vector.reciprocal(vrl[:], sloc[:])
                vloc = scr.tile([128, ND], F32, tag="vloc")
                nc.vector.tensor_scalar_mul(vloc[:], vrl[:], INV_N)
                closs = scr.tile([128, ND], F32, tag="closs")
                nc.vector.tensor_mul(closs[:], uloc_f_last[p][:], vloc[:])

                # row sums of exp(P) with fused accumulate (half-row chunks)
                srow8 = sb.tile([128, 2 * ND], F32, tag=f"srow8{p}")
                for m in range(ND):
                    for half in range(2):
                        pt = scr.tile([128, 2048], BF16, tag="pt", bufs=2)
                        nc.vector.tensor_scalar_mul(
                            pt[:], krow[p][:, N * m + 2048 * half:
                                           N * m + 2048 * (half + 1)],
                            closs[:, m:m + 1])
                        strash = scr.tile([128, 2048], BF16, tag="strash",
                                          bufs=1)
                        nc.scalar.activation(
                            strash[:], pt[:], AF.Exp,
                            accum_out=srow8[:, 2 * m + half:2 * m + half + 1])
                srow = sb.tile([128, ND], F32, tag=f"srow{p}")
                nc.vector.tensor_reduce(
                    srow[:], srow8[:].rearrange("p (m h) -> p m h", h=2),
                    axis=mybir.AxisListType.X, op=mybir.AluOpType.add)
                logs = scr.tile([128, ND], F32, tag="logs")
                nc.scalar.activation(logs[:], srow[:], AF.Ln)
                logred = scr.tile([128, 1], F32, tag="logred")
                nc.vector.reduce_sum(logred[:], logs[:],
                                     axis=mybir.AxisListType.X)
                nc.vector.tensor_add(total[:], total[:], logred[:])
                dterm = scr.tile([128, ND], F32, tag="dterm")
                nc.vector.tensor_mul(dterm[:], closs[:], kdiag[:])
                dred = scr.tile([128, 1], F32, tag="dred")
                nc.vector.reduce_sum(dred[:], dterm[:],
                                     axis=mybir.AxisListType.X)
                nc.vector.tensor_sub(total[:], total[:], dred[:])

            lowp.__exit__(None, None, None)

            # partition sum via ones.T @ total (fp32 matmul, 1 column)
            with tc.tile_pool(name="pssc", bufs=1, space="PSUM") as pssc:
                tot_ps = pssc.tile([1, 1], F32, tag="tot")
                nc.tensor.matmul(tot_ps[:], one_ap, total[:],
                                 start=True, stop=True)
                tot_sb = sb.tile([1, 1], F32, tag="totsb")
                nc.scalar.copy(tot_sb[:], tot_ps[:])

            tar_in = dram.tile([1, 1], F32, tag="tarin")
            tar_out = dram.tile([1, 1], F32, tag="tarout")
            nc.gpsimd.dma_start(tar_in[:], tot_sb[:])
            nc.gpsimd.collective_compute(
                "AllReduce", mybir.AluOpType.add,
                ins=[tar_in[:].opt()], outs=[tar_out[:].opt()],
                replica_groups=RG)
            fin = sb.tile([1, 1], F32, tag="fin")
            nc.sync.dma_start(fin[:], tar_out[:])
            out_sb = sb.tile([1, 1], F32, tag="outsb")
            nc.scalar.mul(out_sb[:], fin[:], HALF_INV_N)
            nc.sync.dma_start(loss_d, out_sb[:])

    nc.compile()
    return nc


_NC_CACHE = {}


def _get_program():
    if "nc" not in _NC_CACHE:
        _NC_CACHE["nc"] = _build_program()
    return _NC_CACHE["nc"]


def kernel(all_image_features, all_text_features, labels=None, **_unused):
    img = np.asarray(all_image_features, dtype=np.float32)
    txt = np.asarray(all_text_features, dtype=np.float32)
    assert img.shape == (N, D) and txt.shape == (N, D)

    # host-side marshaling only: bf16 cast + transpose + per-core slicing
    imgT = np.ascontiguousarray(img.T).astype(NP_BF16)
    txtT = np.ascontiguousarray(txt.T).astype(NP_BF16)
    img_bf = img.astype(NP_BF16)
    txt_bf = txt.astype(NP_BF16)

    in_maps = []
    for c in range(NCORES):
        sl = slice(S * c, S * (c + 1))
        vm = np.zeros((128, ND * NT), dtype=np.float32)
        for q in range(ND):
            vm[:, NT * q + ND * c + q] = 1.0
        in_maps.append({
            "imgT": imgT,
            "txtT": txtT,
            "ilocT": np.ascontiguousarray(imgT[:, sl]),
            "tlocT": np.ascontiguousarray(txtT[:, sl]),
            "iln": np.ascontiguousarray(img_bf[sl, :]),
            "tln": np.ascontiguousarray(txt_bf[sl, :]),
            "vmask": vm,
        })

    nc = _get_program()
    trace = bool(int(os.environ.get("OT_KERNEL_TRACE", "0")))
    res = run_bass_kernel_spmd(nc, in_maps, list(range(NCORES)), trace=trace)
    if trace:
        _NC_CACHE["last_exec_time_ns"] = res.exec_time_ns
        _NC_CACHE["last_results"] = res
    loss = np.float32(res.results[0]["loss"][0, 0])
    return np.asarray(loss, dtype=np.float32).reshape(())
